# revision 38
# baseline (speedup 1.0000x reference)
"""Multi-head attention Bass kernel for Trainium2, 8-core SPMD.

Problem: B=2, S=2048, H=1024, 16 heads of 64 (torch-style MHA without
1/sqrt(d) scaling, key-padding mask, eval mode).

Sharding: core c handles batch b = c//4 and 4 heads (feature slice
256*(c%4) .. +256). Each core computes Q/K/V projections for its feature
slice over its batch, then attention for its 4 heads, producing
out[b, :, fslice] (bf16). Host concatenates and casts to f32.

Key-padding compaction: masked key positions contribute exactly
exp(-1e10) = 0 to softmax, so the host drops masked key/value rows and
pads to a multiple of 128 (typically 1152 of 2048 remain). Padding rows
get the -1e10 bias (folded into the exp() activation of the last kv
tile) so they also contribute 0.

Everything runs "transposed" (feature dim on partitions); activations
arrive pre-transposed and pre-cast to bf16 from the host:
  - Q^T, K^T [f, s] bf16: scores S^T[kpos, q] = K^T.T @ Q^T (contraction
    d=64); the two heads of each 128-feature tile go to different PE
    row groups and run concurrently in the array
  - V kept [s, f] bf16 with an appended ones column per head, so the PV
    matmul gives out[0:64,:] = unnormalized out^T and out[64,:] = the
    softmax denominator
  - small PE transposes of [65, 128] result blocks give [q, 65] where
    normalization is one scalar_tensor_tensor (mul by reciprocal of
    col 64, add output bias) per block.
No max-subtraction in softmax: |scores| <~ 50 for randn-scale inputs,
exp fits f32/bf16 range comfortably.

Engine plan (per the TRN2 cost model):
  - PE: projections + scores + PV + output transposes (~220k cycles,
    the critical engine). A warmup matmul stream covers the initial
    DMA fill and keeps the clock-ramp p-state warm.
  - ACT: only the exp() activations (72 x [128,1024]).
  - Pool (gpsimd): all PSUM->SBUF copies (+ Q/K bias adds, f32->bf16
    casts) -- Pool has no SBUF/PSUM access-latency penalty.
  - DVE: reciprocals + fused normalize (scalar_tensor_tensor).
  - DMA: ~20 large batched transfers (HWDGE costs ~625ns per DMA
    instruction and DMA_ENGINES is one exclusive device, so bytes AND
    instruction count matter). bf16 halves traffic vs f32.
The kt steady state is ACT-bound (~1038ns/kt vs ~852ns of PE work), so
leftover projection/transpose work is drip-fed into the attention kt
loops as one filler unit per slot. Long-lived projection PSUM gets a
dedicated bank (ps_qk) so it never steals a scores rotation buffer;
transpose targets rotate through the accumulator banks after their
oT copies free them.
"""
import numpy as np

import concourse.bass as bass
import concourse.mybir as mybir
import concourse.tile as tile
from concourse.bass_utils import run_bass_kernel_spmd
from concourse.masks import make_identity

B, S, H = 2, 2048, 1024
NH, HD = 16, 64
N_CORES = 8
HPC = NH // (N_CORES // B)   # 4 heads per core
F = HPC * HD                 # 256 features per core
NEG = -10000000000.0

F32 = mybir.dt.float32
BF16 = mybir.dt.bfloat16
I16 = mybir.dt.int16
MMDT = BF16
N_WARM = 0           # initial warmup stream length
N_WARM_MID = (0, 0)  # warmups after K0-proj / after Q0m0-proj
K_SPLIT = True       # 128-wide leading K projection group
LAG00 = 8            # PV lag for the first attention group


def _legalize_sync(nc, max_waits=1, max_updates=1):
    """This walrus build supports at most 1 sync wait / 1 sync update per
    instruction; split excess waits onto preceding same-engine NoOps."""
    n_upd = 0
    for f in nc.m.functions:
        for blk in f.blocks:
            out = []
            for inst in blk.instructions:
                si = getattr(inst, "sync_info", None)
                if si is not None and len(si.on_wait) > max_waits:
                    waits = list(si.on_wait)
                    for k, w in enumerate(waits[:-max_waits]):
                        out.append(mybir.InstNoOp(
                            name=f"{inst.name}-wsplit{k}",
                            sync_info=mybir.SyncInfo(on_wait=[w], on_update=[]),
                            bass_nofuse=True,
                            engine=inst.engine,
                        ))
                    inst.sync_info = mybir.SyncInfo(
                        on_wait=waits[-max_waits:], on_update=list(si.on_update))
                si = getattr(inst, "sync_info", None)
                if si is not None and len(si.on_update) > max_updates:
                    n_upd += 1
                out.append(inst)
            blk.instructions = out
    if n_upd:
        raise RuntimeError(f"{n_upd} instructions need >1 sync updates")


def _groups(total):
    """Split positions into groups of <=512 (PSUM bank = 512 f32)."""
    out = []
    pos = 0
    while pos < total:
        w = min(512, total - pos)
        out.append((pos, w))
        pos += w
    return out


def _groups_k(total):
    """K-projection groups: a small 128-wide leading group so the first
    score matmul can start as soon as possible, then <=512 chunks."""
    if total <= 512:
        return _groups(total)
    return [(0, 128)] + [(128 + p, w) for p, w in _groups(total - 128)]


def _emit(nc, tc, d, s_kv):
    from contextlib import ExitStack
    Exp = mybir.ActivationFunctionType.Exp
    ADD = mybir.AluOpType.add
    MUL = mybir.AluOpType.mult
    NQ = S // 512        # 4 query groups of 512
    NTK = s_kv // 128    # kv tiles of 128
    gk = _groups(s_kv)       # V load/projection groups
    gkk = _groups_k(s_kv) if K_SPLIT else _groups(s_kv)
    gq = _groups(S)

    with ExitStack() as ctx:
        const = ctx.enter_context(tc.tile_pool(name="const", bufs=1))
        # wrm memset first so PE warmup can start immediately; ident and
        # Vt initialization (needed much later) follow the DMA issue.
        wrm = const.tile([128, 256], BF16, tag="wrm", name="wrm")
        nc.gpsimd.memset(wrm[:].bitcast(I16), 0)
        ident = const.tile([128, 128], BF16, tag="ident", name="ident")
        bqk_sb = const.tile([128, 4], F32, tag="bqk", name="bqk")
        mb_sb = const.tile([128, 1], F32, tag="mb", name="mb")
        bvb = const.tile([128, F], F32, tag="bvb", name="bvb")

        qkv = ctx.enter_context(tc.tile_pool(name="qkv", bufs=1))
        # K^T/Q^T kept in float32r: the scores matmul then adds no
        # rounding beyond the bf16 x/w inputs (tf32-level products)
        QKDT = mybir.dt.float32r
        QTt = [[qkv.tile([128, 512], QKDT, tag=f"qt{m}_{g}", name=f"qt{m}_{g}")
                for g in range(NQ)] for m in range(2)]
        KTm = [qkv.tile([128, s_kv], QKDT, tag=f"kt{m}", name=f"kt{m}")
               for m in range(2)]
        # V row-blocks [128, 4*65]: per head 64 value cols + a ones col
        Vt = [qkv.tile([128, HPC * 65], MMDT, tag=f"v{t}", name=f"v{t}")
              for t in range(NTK)]
        outp = [qkv.tile([128, 1024], MMDT, tag=f"out{g}", name=f"out{g}")
                for g in range(NQ)]

        w_p = ctx.enter_context(tc.tile_pool(name="wT", bufs=1))
        xkv_p = ctx.enter_context(tc.tile_pool(name="xkv", bufs=3))
        xq_p = ctx.enter_context(tc.tile_pool(name="xq", bufs=1))
        es_p = ctx.enter_context(tc.tile_pool(name="expS", bufs=11))
        oT_p = ctx.enter_context(tc.tile_pool(name="oT", bufs=3))
        sm_p = ctx.enter_context(tc.tile_pool(name="sm", bufs=4))

        # PSUM: proj 1 + vt/warm 1 + scores 2x2 + acc/pt 2x1 = 8 banks
        ps_qk = ctx.enter_context(
            tc.tile_pool(name="ps_qk", bufs=1, space="PSUM"))
        ps_vt = ctx.enter_context(
            tc.tile_pool(name="ps_vt", bufs=1, space="PSUM"))
        ps_s = ctx.enter_context(
            tc.tile_pool(name="ps_s", bufs=2, space="PSUM"))
        ps_o = ctx.enter_context(
            tc.tile_pool(name="ps_o", bufs=2, space="PSUM"))

        # ---- DMA issue (sync/SP -> HWDGE), in consumption order ----
        w_sb = {nm: w_p.tile([128, 8 * F], MMDT, tag=nm, name=nm)
                for nm in ("wk", "wq", "wv")}

        def load_w(nm):
            nc.sync.dma_start(
                w_sb[nm][:].rearrange("p (c f) -> p c f", c=8),
                d[nm + "T"].rearrange("(c p) f -> p c f", p=128))

        def load_x(x_d, gpos, gw, pool, tag, nsplit=1):
            """nsplit>1 splits the transfer into c-chunk halves so the
            first projection matmuls (which read low c chunks) can start
            before the full group has landed (deps are range-tracked)."""
            xT = pool.tile([128, 8 * 512], MMDT, tag=tag, name=tag)
            xTv = xT[:, 0:8 * gw].rearrange("p (c b) -> p c b", c=8)
            xs = x_d[:, gpos:gpos + gw].rearrange("(c p) b -> p c b", p=128)
            step = 8 // nsplit
            for i in range(0, 8, step):
                nc.sync.dma_start(xTv[:, i:i + step, :], xs[:, i:i + step, :])
            return xTv

        load_w("wk")
        xk_t = [load_x(d["xkT"], gkk[0][0], gkk[0][1], xkv_p, "xkv")]
        load_w("wq")
        xq_t = [load_x(d["xqT"], gq[0][0], gq[0][1], xq_p, "xq0",
                        nsplit=2)]
        # Q/K bias right after the critical prefix (first Pool copy
        # needs it); the rest of the consts are needed much later
        nc.sync.dma_start(bqk_sb[:, 0:2], d["bqr"])
        nc.sync.dma_start(bqk_sb[:, 2:4], d["bkr"])
        xk_t += [load_x(d["xkT"], gp, gw, xkv_p, "xkv", nsplit=2)
                 for gp, gw in gkk[1:2]]
        load_w("wv")
        xk_t += [load_x(d["xkT"], gp, gw, xkv_p, "xkv")
                 for gp, gw in gkk[2:]]
        xv_t = [load_x(d["xvT"], gk[0][0], gk[0][1], xkv_p, "xkv",
                        nsplit=2)]
        nc.sync.dma_start(mb_sb[:], d["mbias"])
        nc.sync.dma_start(bvb[:], d["bvr"].to_broadcast((128, F)))
        xv_t += [load_x(d["xvT"], gp, gw, xkv_p, "xkv") for gp, gw in gk[1:]]
        xq_t += [load_x(d["xqT"], gp, gw, xq_p, f"xq{i + 1}")
                 for i, (gp, gw) in enumerate(gq[1:])]

        # late SBUF initialization (needed only once attention starts)
        make_identity(nc, ident)
        for t in range(NTK):
            nc.gpsimd.memset(Vt[t][:].bitcast(I16), 0x3F80)

        # ---- PE warmup: keeps the PE busy-streak alive through the
        # initial DMA fill so real matmuls evaluate at full clock ----
        def warm(n):
            for i in range(n):
                pw = ps_vt.tile([128, F], F32, tag="pvt", name="pw")
                nc.tensor.matmul(pw[:, 0:256], wrm[:, 0:128], wrm[:],
                                 start=True, stop=True)
        warm(N_WARM)

        # ---- projection emission helpers ----
        def kq_proj(wname, xTv, gw, dst_m, bcol, m):
            """8 matmuls + 1 Pool bias-copy for one (group, m). Uses the
            scores PSUM pool (only called before attention starts)."""
            pq = ps_s.tile([128, 1024], F32, tag="ps", name="pq")
            for c in range(8):
                nc.tensor.matmul(
                    pq[:, 0:gw],
                    w_sb[wname][:, 256 * c + 128 * m:256 * c + 128 * (m + 1)],
                    xTv[:, c, :],
                    start=(c == 0), stop=(c == 7))
            nc.vector.tensor_scalar(dst_m, pq[:, 0:gw],
                                    bqk_sb[:, bcol + m:bcol + m + 1], None,
                                    op0=ADD)

        def kq_units(wname, xTv, gw, dst_m, bcol, m, nunits):
            """Filler closures for one (group, m) projection; PSUM from the
            dedicated long-lived ps_qk bank."""
            box = {}
            per = -(-8 // nunits)
            units = []
            for u in range(nunits):
                cs = list(range(per * u, min(8, per * (u + 1))))

                def emit(cs=cs, last=(u == nunits - 1)):
                    for c in cs:
                        if c == 0:
                            box["pq"] = ps_qk.tile([128, 512], F32,
                                                   tag="pq", name="pq")
                        nc.tensor.matmul(
                            box["pq"][:, 0:gw],
                            w_sb[wname][:, 256 * c + 128 * m:
                                        256 * c + 128 * (m + 1)],
                            xTv[:, c, :],
                            start=(c == 0), stop=(c == 7))
                    if last:
                        nc.vector.tensor_scalar(
                            dst_m, box["pq"][:, 0:gw],
                            bqk_sb[:, bcol + m:bcol + m + 1], None, op0=ADD)
                units.append(emit)
            return units

        def v_unit(j):
            def emit():
                gi = 0
                while 128 * j >= gk[gi][0] + gk[gi][1]:
                    gi += 1
                off = 128 * j - gk[gi][0]
                pv = ps_vt.tile([128, F], F32, tag="pvt", name="pv")
                for c in range(8):
                    nc.tensor.matmul(
                        pv[:],
                        xv_t[gi][:, c, off:off + 128],
                        w_sb["wv"][:, 256 * c:256 * (c + 1)],
                        start=(c == 0), stop=(c == 7))
                nc.vector.tensor_copy(
                    Vt[j][:].rearrange("p (h e) -> p h e", e=65)[:, :, 0:64],
                    pv[:].rearrange("p (h e) -> p h e", h=HPC))
            return emit

        # ---- deferred output transform for a finished (g, m): the oT
        # copies were already emitted inside c_group at accumulator stop;
        # these two filler units do transposes + normalize (+ out DMA) ----
        def t_units(g, m, oT):
            # allocate now (before the next group's accumulators) so the
            # ps_o rotation stays acc0, acc1, pt0, pt1, acc0', ...
            pts = [ps_o.tile([128, 512], MMDT, tag="acc", name="ptt")
                   for _ in range(2)]

            tail = (g == NQ - 1 and m == 1)

            def mk(hh):
                def emit():
                    # each head's transposes land in their own accumulator
                    # bank (freed by the oT copy) -> no head-of-line block
                    pt = pts[hh]
                    h = 2 * m + hh
                    for j in range(4):
                        # 66-col stride keeps each PSUM write 4B-aligned
                        nc.tensor.transpose(
                            pt[:, 66 * j:66 * j + 65],
                            oT[0:65,
                               512 * hh + 128 * j:512 * hh + 128 * (j + 1)],
                            ident[0:65, 0:65])
                    ptv = pt[:, 0:264].rearrange("p (j e) -> p j e", e=66)
                    rc = sm_p.tile([128, 4], F32, tag="rc", name="rc")
                    nc.vector.reciprocal(rc[:], ptv[:, :, 64])
                    # tail group h1: normalize via ACT (scale operand) +
                    # Pool SBUF bias-add, concurrent with h0's DVE chain
                    # (Pool cannot read PSUM, so mid-kernel stays on DVE)
                    for j in range(4):
                        osl = outp[g][:,
                                      256 * j + 64 * h:256 * j + 64 * (h + 1)]
                        if tail and hh == 1:
                            nc.scalar.activation(
                                osl, ptv[:, j, 0:64],
                                mybir.ActivationFunctionType.Identity,
                                scale=rc[:, j:j + 1])
                            nc.gpsimd.tensor_add(
                                osl, osl, bvb[:, 64 * h:64 * (h + 1)])
                        else:
                            nc.vector.scalar_tensor_tensor(
                                osl, ptv[:, j, 0:64], rc[:, j:j + 1],
                                bvb[:, 64 * h:64 * (h + 1)],
                                op0=MUL, op1=ADD)
                        if tail and hh == 1 and j % 2 == 1:
                            # last group: split stores to shorten the tail
                            nc.sync.dma_start(
                                d["out"][512 * g + 128 * (j - 1):
                                         512 * g + 128 * (j + 1), :]
                                .rearrange("(j p) f -> p j f", p=128),
                                outp[g][:, 256 * (j - 1):256 * (j + 1)]
                                .rearrange("p (j f) -> p j f", j=2))
                    if not tail and m == 1 and hh == 1:
                        nc.sync.dma_start(
                            d["out"][512 * g:512 * (g + 1), :].rearrange(
                                "(j p) f -> p j f", p=128),
                            outp[g][:].rearrange("p (j f) -> p j f", j=4))
                return emit
            return [mk(0), mk(1)]

        # ---- attention kt loop for one (g, m) ----
        def c_group(g, m, fillers, lag, lag1=None):
            if lag1 is None:
                lag1 = lag
            acc0 = ps_o.tile([128, 512], F32, tag="acc", name="acc")
            acc1 = ps_o.tile([128, 512], F32, tag="acc", name="acc")
            oT = oT_p.tile([65, 1024], MMDT, tag="oT", name="oT")
            h0, h1 = 2 * m, 2 * m + 1
            esq = []
            for kt in range(max(NTK + lag1, len(fillers))):
                if kt < NTK:
                    ksl = slice(128 * kt, 128 * (kt + 1))
                    ps = ps_s.tile([128, 1024], F32, tag="ps", name="ps")
                    nc.tensor.matmul(
                        ps[:, 0:512], KTm[m][0:64, ksl], QTt[m][g][0:64, :],
                        start=True, stop=True)
                    nc.tensor.matmul(
                        ps[:, 512:1024], KTm[m][64:128, ksl],
                        QTt[m][g][64:128, :],
                        start=True, stop=True)
                    es = es_p.tile([128, 1024], MMDT, tag="es", name="es")
                    if kt == NTK - 1:
                        nc.scalar.activation(es[:], ps[:], Exp,
                                             bias=mb_sb[:, 0:1])
                    else:
                        nc.scalar.activation(es[:], ps[:], Exp)
                    esq.append(es)
                if kt < len(fillers):
                    for u in (fillers[kt] or []):
                        u()
                pk = kt - lag
                if 0 <= pk < NTK:
                    nc.tensor.matmul(
                        acc0[0:65, :], Vt[pk][:, 65 * h0:65 * (h0 + 1)],
                        esq[pk][:, 0:512],
                        start=(pk == 0), stop=(pk == NTK - 1))
                    if pk == NTK - 1:
                        nc.vector.tensor_copy(oT[0:65, 0:512], acc0[0:65, :])
                pk = kt - lag1
                if 0 <= pk < NTK:
                    nc.tensor.matmul(
                        acc1[0:65, :], Vt[pk][:, 65 * h1:65 * (h1 + 1)],
                        esq[pk][:, 512:1024],
                        start=(pk == 0), stop=(pk == NTK - 1))
                    if pk == NTK - 1:
                        if lag1 != lag:
                            # tail group: ACT is idle by now -- run this
                            # copy there, concurrent with acc0's DVE copy
                            nc.scalar.activation(
                                oT[0:65, 512:1024], acc1[0:65, :],
                                mybir.ActivationFunctionType.Identity)
                        else:
                            nc.vector.tensor_copy(oT[0:65, 512:1024],
                                                  acc1[0:65, :])
            return oT

        # ---- up-front projections: K group 0 (both m) and Q0 (m=0),
        # with warmup bursts bridging their DMA waits ----
        for m in range(2):
            gp, gw = gkk[0]
            kq_proj("wk", xk_t[0], gw, KTm[m][:, gp:gp + gw], 2, m)
        warm(N_WARM_MID[0])
        kq_proj("wq", xq_t[0], 512, QTt[0][0], 0, 0)
        warm(N_WARM_MID[1])

        # ---- filler schedules ----
        vu = [v_unit(j) for j in range(NTK)]
        q0m1 = kq_units("wq", xq_t[0], 512, QTt[1][0], 0, 1, 2)
        km = {(gi, m): kq_units("wk", xk_t[gi], gkk[gi][1],
                                KTm[m][:, gkk[gi][0]:gkk[gi][0] + gkk[gi][1]],
                                2, m, 2 if gkk[gi][1] > 256 else 1)
              for gi in range(1, len(gkk)) for m in range(2)}
        qum = {(g, m): kq_units("wq", xq_t[g], 512, QTt[m][g], 0, m, 9)
               for g in range(1, NQ) for m in range(2)}

        # (0,0) slot schedule (lists per slot). K-group gi covers kv
        # tiles starting at gkk[gi][0]//128, so its m0 copy must be
        # emitted before that scores slot; m1 copies before (0,1).
        if len(gkk) == 3:
            f00 = [km[(1, 0)], q0m1[:1], q0m1[1:],
                   km[(2, 0)][:1], km[(2, 0)][1:],
                   km[(1, 1)][:1], km[(1, 1)][1:],
                   [vu[0]], [vu[1]], [vu[2]], [vu[3]],
                   [vu[4]] + km[(2, 1)][:1], [vu[5]] + km[(2, 1)][1:],
                   [vu[6]], [vu[7]], [vu[8]]]
            lag00 = LAG00
        else:
            # generic fallback: all projection units first, then V units
            rest = [u for gi in range(1, len(gkk))
                    for mm in range(2) for u in km[(gi, mm)]] + q0m1
            f00 = [[u] for u in rest] + [[u] for u in vu]
            lag00 = max(len(f00) - NTK, len(rest) + 1, 2)
        oTs = {}
        oTs[(0, 0)] = c_group(0, 0, f00, lag00)

        order = [(g, m) for g in range(NQ) for m in range(2)][1:]
        prev = (0, 0)
        for (g, m) in order:
            tu = t_units(prev[0], prev[1], oTs[prev])
            if (g, m) == (0, 1):
                rest = qum[(1, 0)]
            elif m == 0:
                rest = qum[(g, 1)]
            elif g < NQ - 1:
                rest = qum[(g + 1, 0)]
            else:
                rest = [None, None]
            fl = rest[:4] + tu + rest[4:]
            tpos = min(4, len(rest))
            lag = max(len(fl) - NTK, tpos + 2, 2)
            # tail group: stagger the second accumulator so the two
            # heads' output chains overlap at the very end
            lag1 = lag + 2 if (g, m) == (NQ - 1, 1) else None
            oTs[(g, m)] = c_group(g, m, [[u] if u else None for u in fl],
                                  lag, lag1)
            prev = (g, m)
        # tail: last group's output transform, hand-scheduled so the two
        # heads' normalize chains run on DVE and ACT/Pool concurrently
        g, oT = NQ - 1, oTs[(NQ - 1, 1)]
        pts = [ps_o.tile([128, 512], MMDT, tag="acc", name="ptt")
               for _ in range(2)]
        rcs = [sm_p.tile([128, 4], F32, tag="rc", name="rc")
               for _ in range(2)]
        ptvs = []
        for hh in range(2):
            for j in range(4):
                nc.tensor.transpose(
                    pts[hh][:, 66 * j:66 * j + 65],
                    oT[0:65, 512 * hh + 128 * j:512 * hh + 128 * (j + 1)],
                    ident[0:65, 0:65])
            ptv = pts[hh][:, 0:264].rearrange("p (j e) -> p j e", e=66)
            ptvs.append(ptv)
            nc.vector.reciprocal(rcs[hh][:], ptv[:, :, 64])
        for j in range(4):
            for hh in range(2):
                h = 2 + hh
                osl = outp[g][:, 256 * j + 64 * h:256 * j + 64 * (h + 1)]
                if hh == 1:
                    nc.scalar.activation(
                        osl, ptvs[hh][:, j, 0:64],
                        mybir.ActivationFunctionType.Identity,
                        scale=rcs[hh][:, j:j + 1])
                    nc.gpsimd.tensor_add(osl, osl,
                                         bvb[:, 64 * h:64 * (h + 1)])
                else:
                    nc.vector.scalar_tensor_tensor(
                        osl, ptvs[hh][:, j, 0:64], rcs[hh][:, j:j + 1],
                        bvb[:, 64 * h:64 * (h + 1)], op0=MUL, op1=ADD)
            if j % 2 == 1:
                nc.sync.dma_start(
                    d["out"][512 * g + 128 * (j - 1):
                             512 * g + 128 * (j + 1), :]
                    .rearrange("(j p) f -> p j f", p=128),
                    outp[g][:, 256 * (j - 1):256 * (j + 1)]
                    .rearrange("p (j f) -> p j f", j=2))


_NC_CACHE = {}


def _build(s_kv):
    if s_kv in _NC_CACHE:
        return _NC_CACHE[s_kv]
    nc = bass.Bass(trn_type="TRN2", target_bir_lowering=False, debug=False)
    d = {
        "xqT": nc.dram_tensor("xqT", [H, S], MMDT, kind="ExternalInput").ap(),
        "xkT": nc.dram_tensor("xkT", [H, s_kv], MMDT,
                              kind="ExternalInput").ap(),
        "xvT": nc.dram_tensor("xvT", [H, s_kv], MMDT,
                              kind="ExternalInput").ap(),
        "wqT": nc.dram_tensor("wqT", [H, F], MMDT, kind="ExternalInput").ap(),
        "wkT": nc.dram_tensor("wkT", [H, F], MMDT, kind="ExternalInput").ap(),
        "wvT": nc.dram_tensor("wvT", [H, F], MMDT, kind="ExternalInput").ap(),
        "bqr": nc.dram_tensor("bqr", [128, 2], F32, kind="ExternalInput").ap(),
        "bkr": nc.dram_tensor("bkr", [128, 2], F32, kind="ExternalInput").ap(),
        "bvr": nc.dram_tensor("bvr", [1, F], F32, kind="ExternalInput").ap(),
        "mbias": nc.dram_tensor("mbias", [128, 1], F32,
                                kind="ExternalInput").ap(),
        "out": nc.dram_tensor("out", [S, F], MMDT, kind="ExternalOutput").ap(),
    }
    with tile.TileContext(nc) as tc:
        _emit(nc, tc, d, s_kv)
    _legalize_sync(nc)
    _NC_CACHE[s_kv] = nc
    return nc


def plan_kv(mask):
    """Per-batch compaction plan: indices of valid key positions and the
    padded kv length shared across batches (multiple of 128)."""
    mask = np.asarray(mask)
    idxs = [np.nonzero(mask[b])[0] for b in range(B)]
    nmax = max((len(i) for i in idxs), default=1)
    s_kv = min(S, max(128, -(-nmax // 128) * 128))
    return idxs, s_kv


def make_in_maps(query, key, value, mask, Wq, bq, Wk, bk, Wv, bv,
                 idxs=None, s_kv=None):
    import ml_dtypes
    bf16 = ml_dtypes.bfloat16
    if idxs is None:
        idxs, s_kv = plan_kv(mask)
    query, key, value = (np.asarray(a, np.float32)
                         for a in (query, key, value))
    Wq, Wk, Wv = (np.asarray(a, np.float32) for a in (Wq, Wk, Wv))
    bq, bk, bv = (np.asarray(a, np.float32) for a in (bq, bk, bv))
    in_maps = []
    qc, kc, vc, mbc = {}, {}, {}, {}
    for b in range(B):
        idx = idxs[b]
        qc[b] = np.ascontiguousarray(query[b].T.astype(bf16))
        kcb = np.zeros((H, s_kv), bf16)
        kcb[:, :len(idx)] = key[b][idx].T.astype(bf16)
        vcb = np.zeros((H, s_kv), bf16)
        vcb[:, :len(idx)] = value[b][idx].T.astype(bf16)
        # per-partition bias column for the LAST kv tile only
        mb = np.full(128, NEG, np.float32)
        nlast = len(idx) - (s_kv - 128)
        if nlast > 0:
            mb[:nlast] = 0.0
        kc[b], vc[b] = kcb, vcb
        mbc[b] = np.ascontiguousarray(mb.reshape(128, 1))
    for c in range(N_CORES):
        b = c // (N_CORES // B)
        fs = F * (c % (N_CORES // B))
        in_maps.append({
            "xqT": qc[b],
            "xkT": kc[b],
            "xvT": vc[b],
            "wqT": np.ascontiguousarray(Wq[fs:fs + F].T.astype(bf16)),
            "wkT": np.ascontiguousarray(Wk[fs:fs + F].T.astype(bf16)),
            "wvT": np.ascontiguousarray(Wv[fs:fs + F].T.astype(bf16)),
            "bqr": np.ascontiguousarray(bq[fs:fs + F].reshape(2, 128).T),
            "bkr": np.ascontiguousarray(bk[fs:fs + F].reshape(2, 128).T),
            "bvr": np.ascontiguousarray(bv[fs:fs + F].reshape(1, F)),
            "mbias": mbc[b],
        })
    return in_maps


def assemble(results):
    out = np.empty((B, S, H), np.float32)
    for c in range(N_CORES):
        b = c // (N_CORES // B)
        fs = F * (c % (N_CORES // B))
        out[b, :, fs:fs + F] = np.asarray(results[c]["out"],
                                          dtype=np.float32)
    return out


def kernel(query, key, value, mask, Wq, bq, Wk, bk, Wv, bv, _trace=False):
    idxs, s_kv = plan_kv(mask)
    nc = _build(s_kv)
    in_maps = make_in_maps(query, key, value, mask, Wq, bq, Wk, bk, Wv, bv,
                           idxs, s_kv)
    res = run_bass_kernel_spmd(nc, in_maps, core_ids=list(range(N_CORES)),
                               trace=_trace)
    out = assemble(res.results)
    if _trace:
        return out, res
    return out


# revision 41
# speedup vs baseline: 1.0030x; 1.0030x over previous
"""Multi-head attention Bass kernel for Trainium2, 8-core SPMD.

Problem: B=2, S=2048, H=1024, 16 heads of 64 (torch-style MHA without
1/sqrt(d) scaling, key-padding mask, eval mode).

Sharding: core c handles batch b = c//4 and 4 heads (feature slice
256*(c%4) .. +256). Each core computes Q/K/V projections for its feature
slice over its batch, then attention for its 4 heads, producing
out[b, :, fslice] (bf16). Host concatenates and casts to f32.

Key-padding compaction: masked key positions contribute exactly
exp(-1e10) = 0 to softmax, so the host drops masked key/value rows and
pads to a multiple of 128 (typically 1152 of 2048 remain). Padding rows
get the -1e10 bias (folded into the exp() activation of the last kv
tile) so they also contribute 0.

Everything runs "transposed" (feature dim on partitions); activations
arrive pre-transposed and pre-cast to bf16 from the host:
  - Q^T, K^T [f, s] bf16: scores S^T[kpos, q] = K^T.T @ Q^T (contraction
    d=64); the two heads of each 128-feature tile go to different PE
    row groups and run concurrently in the array
  - V kept [s, f] bf16 with an appended ones column per head, so the PV
    matmul gives out[0:64,:] = unnormalized out^T and out[64,:] = the
    softmax denominator
  - small PE transposes of [65, 128] result blocks give [q, 65] where
    normalization is one scalar_tensor_tensor (mul by reciprocal of
    col 64, add output bias) per block.
No max-subtraction in softmax: |scores| <~ 50 for randn-scale inputs,
exp fits f32/bf16 range comfortably.

Engine plan (per the TRN2 cost model; note GPSIMD/Pool cannot touch
PSUM on real HW, and PSUM stores must be 4B-aligned):
  - PE: projections + scores + PV + output transposes (~220k cycles at
    1 cycle/row in bf16 -- the critical engine, ~93us busy).
  - ACT: the exp() activations (72 x [128,1024], ~75us) + tail copies.
  - DVE: all PSUM->SBUF copies (Q/K bias adds, V interleave, oT),
    reciprocals, fused normalize (scalar_tensor_tensor).
  - Pool (gpsimd): SBUF-only memsets/identity + tail bias add.
  - DMA: ~20 large batched transfers (HWDGE costs ~625ns per DMA
    instruction and DMA_ENGINES is one exclusive device, so bytes AND
    instruction count matter). bf16 halves traffic vs f32.
The kt steady state is ACT-bound (~1038ns/kt vs ~852ns of PE work), so
leftover projection/transpose work is drip-fed into the attention kt
loops as filler units per slot. Long-lived projection PSUM gets a
dedicated bank (ps_qk) so it never steals a scores rotation buffer;
transpose targets rotate through the accumulator banks after their
oT copies free them. No matmul warmups: all real matmuls are
dependency-gated, which the cost model's p-state ramp rewards.
"""
import numpy as np

import concourse.bass as bass
import concourse.mybir as mybir
import concourse.tile as tile
from concourse.bass_utils import run_bass_kernel_spmd
from concourse.masks import make_identity

B, S, H = 2, 2048, 1024
NH, HD = 16, 64
N_CORES = 8
HPC = NH // (N_CORES // B)   # 4 heads per core
F = HPC * HD                 # 256 features per core
NEG = -10000000000.0

F32 = mybir.dt.float32
BF16 = mybir.dt.bfloat16
I16 = mybir.dt.int16
MMDT = BF16
N_WARM = 0           # initial warmup stream length
N_WARM_MID = (0, 0)  # warmups after K0-proj / after Q0m0-proj
K_SPLIT = True       # 128-wide leading K projection group
LAG00 = 8            # PV lag for the first attention group


def _legalize_sync(nc, max_waits=1, max_updates=1):
    """This walrus build supports at most 1 sync wait / 1 sync update per
    instruction; split excess waits onto preceding same-engine NoOps."""
    n_upd = 0
    for f in nc.m.functions:
        for blk in f.blocks:
            out = []
            for inst in blk.instructions:
                si = getattr(inst, "sync_info", None)
                if si is not None and len(si.on_wait) > max_waits:
                    waits = list(si.on_wait)
                    for k, w in enumerate(waits[:-max_waits]):
                        out.append(mybir.InstNoOp(
                            name=f"{inst.name}-wsplit{k}",
                            sync_info=mybir.SyncInfo(on_wait=[w], on_update=[]),
                            bass_nofuse=True,
                            engine=inst.engine,
                        ))
                    inst.sync_info = mybir.SyncInfo(
                        on_wait=waits[-max_waits:], on_update=list(si.on_update))
                si = getattr(inst, "sync_info", None)
                if si is not None and len(si.on_update) > max_updates:
                    n_upd += 1
                out.append(inst)
            blk.instructions = out
    if n_upd:
        raise RuntimeError(f"{n_upd} instructions need >1 sync updates")


def _groups(total):
    """Split positions into groups of <=512 (PSUM bank = 512 f32)."""
    out = []
    pos = 0
    while pos < total:
        w = min(512, total - pos)
        out.append((pos, w))
        pos += w
    return out


def _groups_k(total):
    """K-projection groups: a small 128-wide leading group so the first
    score matmul can start as soon as possible, then <=512 chunks."""
    if total <= 512:
        return _groups(total)
    return [(0, 128)] + [(128 + p, w) for p, w in _groups(total - 128)]


def _emit(nc, tc, d, s_kv):
    from contextlib import ExitStack
    Exp = mybir.ActivationFunctionType.Exp
    ADD = mybir.AluOpType.add
    MUL = mybir.AluOpType.mult
    NQ = S // 512        # 4 query groups of 512
    NTK = s_kv // 128    # kv tiles of 128
    gk = _groups(s_kv)       # V load/projection groups
    gkk = _groups_k(s_kv) if K_SPLIT else _groups(s_kv)
    gq = _groups(S)

    with ExitStack() as ctx:
        const = ctx.enter_context(tc.tile_pool(name="const", bufs=1))
        # wrm memset first so PE warmup can start immediately; ident and
        # Vt initialization (needed much later) follow the DMA issue.
        wrm = const.tile([128, 256], BF16, tag="wrm", name="wrm")
        nc.gpsimd.memset(wrm[:].bitcast(I16), 0)
        ident = const.tile([128, 128], BF16, tag="ident", name="ident")
        bqk_sb = const.tile([128, 4], F32, tag="bqk", name="bqk")
        mb_sb = const.tile([128, 1], F32, tag="mb", name="mb")
        bvb = const.tile([128, F], F32, tag="bvb", name="bvb")

        qkv = ctx.enter_context(tc.tile_pool(name="qkv", bufs=1))
        # K^T/Q^T kept in float32r: the scores matmul then adds no
        # rounding beyond the bf16 x/w inputs (tf32-level products)
        QKDT = mybir.dt.float32r
        QTt = [[qkv.tile([128, 512], QKDT, tag=f"qt{m}_{g}", name=f"qt{m}_{g}")
                for g in range(NQ)] for m in range(2)]
        KTm = [qkv.tile([128, s_kv], QKDT, tag=f"kt{m}", name=f"kt{m}")
               for m in range(2)]
        # V row-blocks [128, 4*65]: per head 64 value cols + a ones col
        Vt = [qkv.tile([128, HPC * 65], MMDT, tag=f"v{t}", name=f"v{t}")
              for t in range(NTK)]
        outp = [qkv.tile([128, 1024], MMDT, tag=f"out{g}", name=f"out{g}")
                for g in range(NQ)]

        w_p = ctx.enter_context(tc.tile_pool(name="wT", bufs=1))
        xkv_p = ctx.enter_context(tc.tile_pool(name="xkv", bufs=3))
        xq_p = ctx.enter_context(tc.tile_pool(name="xq", bufs=1))
        es_p = ctx.enter_context(tc.tile_pool(name="expS", bufs=11))
        oT_p = ctx.enter_context(tc.tile_pool(name="oT", bufs=3))
        sm_p = ctx.enter_context(tc.tile_pool(name="sm", bufs=4))

        # PSUM: proj 1 + vt/warm 1 + scores 2x2 + acc/pt 2x1 = 8 banks
        ps_qk = ctx.enter_context(
            tc.tile_pool(name="ps_qk", bufs=1, space="PSUM"))
        ps_vt = ctx.enter_context(
            tc.tile_pool(name="ps_vt", bufs=1, space="PSUM"))
        ps_s = ctx.enter_context(
            tc.tile_pool(name="ps_s", bufs=2, space="PSUM"))
        ps_o = ctx.enter_context(
            tc.tile_pool(name="ps_o", bufs=2, space="PSUM"))

        # ---- DMA issue (sync/SP -> HWDGE), in consumption order ----
        w_sb = {nm: w_p.tile([128, 8 * F], MMDT, tag=nm, name=nm)
                for nm in ("wk", "wq", "wv")}

        def load_w(nm):
            nc.sync.dma_start(
                w_sb[nm][:].rearrange("p (c f) -> p c f", c=8),
                d[nm + "T"].rearrange("(c p) f -> p c f", p=128))

        def load_x(x_d, gpos, gw, pool, tag, nsplit=1):
            """nsplit>1 splits the transfer into c-chunk halves so the
            first projection matmuls (which read low c chunks) can start
            before the full group has landed (deps are range-tracked)."""
            xT = pool.tile([128, 8 * 512], MMDT, tag=tag, name=tag)
            xTv = xT[:, 0:8 * gw].rearrange("p (c b) -> p c b", c=8)
            xs = x_d[:, gpos:gpos + gw].rearrange("(c p) b -> p c b", p=128)
            step = 8 // nsplit
            for i in range(0, 8, step):
                nc.sync.dma_start(xTv[:, i:i + step, :], xs[:, i:i + step, :])
            return xTv

        load_w("wk")
        xk_t = [load_x(d["xkT"], gkk[0][0], gkk[0][1], xkv_p, "xkv")]
        load_w("wq")
        xq_t = [load_x(d["xqT"], gq[0][0], gq[0][1], xq_p, "xq0")]
        # Q/K bias right after the critical prefix (first Pool copy
        # needs it); the rest of the consts are needed much later
        nc.sync.dma_start(bqk_sb[:, 0:2], d["bqr"])
        nc.sync.dma_start(bqk_sb[:, 2:4], d["bkr"])
        xk_t += [load_x(d["xkT"], gp, gw, xkv_p, "xkv")
                 for gp, gw in gkk[1:2]]
        load_w("wv")
        xk_t += [load_x(d["xkT"], gp, gw, xkv_p, "xkv")
                 for gp, gw in gkk[2:]]
        xv_t = [load_x(d["xvT"], gk[0][0], gk[0][1], xkv_p, "xkv")]
        nc.sync.dma_start(mb_sb[:], d["mbias"])
        nc.sync.dma_start(bvb[:], d["bvr"].to_broadcast((128, F)))
        xv_t += [load_x(d["xvT"], gp, gw, xkv_p, "xkv") for gp, gw in gk[1:]]
        xq_t += [load_x(d["xqT"], gp, gw, xq_p, f"xq{i + 1}")
                 for i, (gp, gw) in enumerate(gq[1:])]

        # late SBUF initialization (needed only once attention starts)
        make_identity(nc, ident)
        for t in range(NTK):
            nc.gpsimd.memset(Vt[t][:].bitcast(I16), 0x3F80)

        # ---- PE warmup: keeps the PE busy-streak alive through the
        # initial DMA fill so real matmuls evaluate at full clock ----
        def warm(n):
            for i in range(n):
                pw = ps_vt.tile([128, F], F32, tag="pvt", name="pw")
                nc.tensor.matmul(pw[:, 0:256], wrm[:, 0:128], wrm[:],
                                 start=True, stop=True)
        warm(N_WARM)

        # ---- projection emission helpers ----
        def kq_proj(wname, xTv, gw, dst_m, bcol, m):
            """8 matmuls + 1 Pool bias-copy for one (group, m). Uses the
            scores PSUM pool (only called before attention starts)."""
            pq = ps_s.tile([128, 1024], F32, tag="ps", name="pq")
            for c in range(8):
                nc.tensor.matmul(
                    pq[:, 0:gw],
                    w_sb[wname][:, 256 * c + 128 * m:256 * c + 128 * (m + 1)],
                    xTv[:, c, :],
                    start=(c == 0), stop=(c == 7))
            nc.vector.tensor_scalar(dst_m, pq[:, 0:gw],
                                    bqk_sb[:, bcol + m:bcol + m + 1], None,
                                    op0=ADD)

        def kq_units(wname, xTv, gw, dst_m, bcol, m, nunits):
            """Filler closures for one (group, m) projection; PSUM from the
            dedicated long-lived ps_qk bank."""
            box = {}
            per = -(-8 // nunits)
            units = []
            for u in range(nunits):
                cs = list(range(per * u, min(8, per * (u + 1))))

                def emit(cs=cs, last=(u == nunits - 1)):
                    for c in cs:
                        if c == 0:
                            box["pq"] = ps_qk.tile([128, 512], F32,
                                                   tag="pq", name="pq")
                        nc.tensor.matmul(
                            box["pq"][:, 0:gw],
                            w_sb[wname][:, 256 * c + 128 * m:
                                        256 * c + 128 * (m + 1)],
                            xTv[:, c, :],
                            start=(c == 0), stop=(c == 7))
                    if last:
                        nc.vector.tensor_scalar(
                            dst_m, box["pq"][:, 0:gw],
                            bqk_sb[:, bcol + m:bcol + m + 1], None, op0=ADD)
                units.append(emit)
            return units

        def v_unit(j):
            def emit():
                gi = 0
                while 128 * j >= gk[gi][0] + gk[gi][1]:
                    gi += 1
                off = 128 * j - gk[gi][0]
                pv = ps_vt.tile([128, F], F32, tag="pvt", name="pv")
                for c in range(8):
                    nc.tensor.matmul(
                        pv[:],
                        xv_t[gi][:, c, off:off + 128],
                        w_sb["wv"][:, 256 * c:256 * (c + 1)],
                        start=(c == 0), stop=(c == 7))
                nc.vector.tensor_copy(
                    Vt[j][:].rearrange("p (h e) -> p h e", e=65)[:, :, 0:64],
                    pv[:].rearrange("p (h e) -> p h e", h=HPC))
            return emit

        # ---- deferred output transform for a finished (g, m): the oT
        # copies were already emitted inside c_group at accumulator stop;
        # these two filler units do transposes + normalize (+ out DMA) ----
        def t_units(g, m, oT):
            # allocate now (before the next group's accumulators) so the
            # ps_o rotation stays acc0, acc1, pt0, pt1, acc0', ...
            pts = [ps_o.tile([128, 512], MMDT, tag="acc", name="ptt")
                   for _ in range(2)]

            tail = (g == NQ - 1 and m == 1)

            def mk(hh):
                def emit():
                    # each head's transposes land in their own accumulator
                    # bank (freed by the oT copy) -> no head-of-line block
                    pt = pts[hh]
                    h = 2 * m + hh
                    for j in range(4):
                        # 66-col stride keeps each PSUM write 4B-aligned
                        nc.tensor.transpose(
                            pt[:, 66 * j:66 * j + 65],
                            oT[0:65,
                               512 * hh + 128 * j:512 * hh + 128 * (j + 1)],
                            ident[0:65, 0:65])
                    ptv = pt[:, 0:264].rearrange("p (j e) -> p j e", e=66)
                    rc = sm_p.tile([128, 4], F32, tag="rc", name="rc")
                    nc.vector.reciprocal(rc[:], ptv[:, :, 64])
                    # tail group h1: normalize via ACT (scale operand) +
                    # Pool SBUF bias-add, concurrent with h0's DVE chain
                    # (Pool cannot read PSUM, so mid-kernel stays on DVE)
                    for j in range(4):
                        osl = outp[g][:,
                                      256 * j + 64 * h:256 * j + 64 * (h + 1)]
                        if tail and hh == 1:
                            nc.scalar.activation(
                                osl, ptv[:, j, 0:64],
                                mybir.ActivationFunctionType.Identity,
                                scale=rc[:, j:j + 1])
                            nc.gpsimd.tensor_add(
                                osl, osl, bvb[:, 64 * h:64 * (h + 1)])
                        else:
                            nc.vector.scalar_tensor_tensor(
                                osl, ptv[:, j, 0:64], rc[:, j:j + 1],
                                bvb[:, 64 * h:64 * (h + 1)],
                                op0=MUL, op1=ADD)
                        if tail and hh == 1 and j % 2 == 1:
                            # last group: split stores to shorten the tail
                            nc.sync.dma_start(
                                d["out"][512 * g + 128 * (j - 1):
                                         512 * g + 128 * (j + 1), :]
                                .rearrange("(j p) f -> p j f", p=128),
                                outp[g][:, 256 * (j - 1):256 * (j + 1)]
                                .rearrange("p (j f) -> p j f", j=2))
                    if not tail and m == 1 and hh == 1:
                        nc.sync.dma_start(
                            d["out"][512 * g:512 * (g + 1), :].rearrange(
                                "(j p) f -> p j f", p=128),
                            outp[g][:].rearrange("p (j f) -> p j f", j=4))
                return emit
            return [mk(0), mk(1)]

        # ---- attention kt loop for one (g, m) ----
        def c_group(g, m, fillers, lag, lag1=None):
            if lag1 is None:
                lag1 = lag
            acc0 = ps_o.tile([128, 512], F32, tag="acc", name="acc")
            acc1 = ps_o.tile([128, 512], F32, tag="acc", name="acc")
            oT = oT_p.tile([65, 1024], MMDT, tag="oT", name="oT")
            h0, h1 = 2 * m, 2 * m + 1
            esq = []
            for kt in range(max(NTK + lag1, len(fillers))):
                if kt < NTK:
                    ksl = slice(128 * kt, 128 * (kt + 1))
                    ps = ps_s.tile([128, 1024], F32, tag="ps", name="ps")
                    nc.tensor.matmul(
                        ps[:, 0:512], KTm[m][0:64, ksl], QTt[m][g][0:64, :],
                        start=True, stop=True)
                    nc.tensor.matmul(
                        ps[:, 512:1024], KTm[m][64:128, ksl],
                        QTt[m][g][64:128, :],
                        start=True, stop=True)
                    es = es_p.tile([128, 1024], MMDT, tag="es", name="es")
                    if kt == NTK - 1:
                        nc.scalar.activation(es[:], ps[:], Exp,
                                             bias=mb_sb[:, 0:1])
                    else:
                        nc.scalar.activation(es[:], ps[:], Exp)
                    esq.append(es)
                if kt < len(fillers):
                    for u in (fillers[kt] or []):
                        u()
                pk = kt - lag
                if 0 <= pk < NTK:
                    nc.tensor.matmul(
                        acc0[0:65, :], Vt[pk][:, 65 * h0:65 * (h0 + 1)],
                        esq[pk][:, 0:512],
                        start=(pk == 0), stop=(pk == NTK - 1))
                    if pk == NTK - 1:
                        nc.vector.tensor_copy(oT[0:65, 0:512], acc0[0:65, :])
                pk = kt - lag1
                if 0 <= pk < NTK:
                    nc.tensor.matmul(
                        acc1[0:65, :], Vt[pk][:, 65 * h1:65 * (h1 + 1)],
                        esq[pk][:, 512:1024],
                        start=(pk == 0), stop=(pk == NTK - 1))
                    if pk == NTK - 1:
                        if lag1 != lag:
                            # tail group: ACT is idle by now -- run this
                            # copy there, concurrent with acc0's DVE copy
                            nc.scalar.activation(
                                oT[0:65, 512:1024], acc1[0:65, :],
                                mybir.ActivationFunctionType.Identity)
                        else:
                            nc.vector.tensor_copy(oT[0:65, 512:1024],
                                                  acc1[0:65, :])
            return oT

        # ---- up-front projections: K group 0 (both m) and Q0 (m=0),
        # with warmup bursts bridging their DMA waits ----
        for m in range(2):
            gp, gw = gkk[0]
            kq_proj("wk", xk_t[0], gw, KTm[m][:, gp:gp + gw], 2, m)
        warm(N_WARM_MID[0])
        kq_proj("wq", xq_t[0], 512, QTt[0][0], 0, 0)
        warm(N_WARM_MID[1])

        # ---- filler schedules ----
        vu = [v_unit(j) for j in range(NTK)]
        q0m1 = kq_units("wq", xq_t[0], 512, QTt[1][0], 0, 1, 2)
        km = {(gi, m): kq_units("wk", xk_t[gi], gkk[gi][1],
                                KTm[m][:, gkk[gi][0]:gkk[gi][0] + gkk[gi][1]],
                                2, m, 2 if gkk[gi][1] > 256 else 1)
              for gi in range(1, len(gkk)) for m in range(2)}
        qum = {(g, m): kq_units("wq", xq_t[g], 512, QTt[m][g], 0, m, 9)
               for g in range(1, NQ) for m in range(2)}

        # (0,0) slot schedule (lists per slot). K-group gi covers kv
        # tiles starting at gkk[gi][0]//128, so its m0 copy must be
        # emitted before that scores slot; m1 copies before (0,1).
        if len(gkk) == 3 and NTK == 9:
            f00 = [km[(1, 0)], q0m1[:1], q0m1[1:],
                   km[(2, 0)][:1], km[(2, 0)][1:],
                   km[(1, 1)][:1], km[(1, 1)][1:],
                   [vu[0]], [vu[1]], [vu[2]], [vu[3]],
                   [vu[4]] + km[(2, 1)][:1], [vu[5]] + km[(2, 1)][1:],
                   [vu[6]], [vu[7]], [vu[8]]]
            lag00 = LAG00
        else:
            # generic fallback (other mask shapes): emit ALL remaining
            # projections inline before attention -- slower but safe
            # against any scores-before-copy emission-order hazard
            for gi in range(1, len(gkk)):
                for mm in range(2):
                    for u in km[(gi, mm)]:
                        u()
            for u in q0m1:
                u()
            f00 = [[u] for u in vu]
            lag00 = max(min(len(vu), 4), 2)
        oTs = {}
        oTs[(0, 0)] = c_group(0, 0, f00, lag00)

        order = [(g, m) for g in range(NQ) for m in range(2)][1:]
        prev = (0, 0)
        for (g, m) in order:
            tu = t_units(prev[0], prev[1], oTs[prev])
            if (g, m) == (0, 1):
                rest = qum[(1, 0)]
            elif m == 0:
                rest = qum[(g, 1)]
            elif g < NQ - 1:
                rest = qum[(g + 1, 0)]
            else:
                rest = [None, None]
            fl = rest[:4] + tu + rest[4:]
            tpos = min(4, len(rest))
            lag = max(len(fl) - NTK, tpos + 2, 2)
            # tail group: stagger the second accumulator so the two
            # heads' output chains overlap at the very end
            lag1 = lag + 2 if (g, m) == (NQ - 1, 1) else None
            oTs[(g, m)] = c_group(g, m, [[u] if u else None for u in fl],
                                  lag, lag1)
            prev = (g, m)
        # tail: last group's output transform, hand-scheduled so the two
        # heads' normalize chains run on DVE and ACT/Pool concurrently
        g, oT = NQ - 1, oTs[(NQ - 1, 1)]
        pts = [ps_o.tile([128, 512], MMDT, tag="acc", name="ptt")
               for _ in range(2)]
        rcs = [sm_p.tile([128, 4], F32, tag="rc", name="rc")
               for _ in range(2)]
        ptvs = []
        for hh in range(2):
            for j in range(4):
                nc.tensor.transpose(
                    pts[hh][:, 66 * j:66 * j + 65],
                    oT[0:65, 512 * hh + 128 * j:512 * hh + 128 * (j + 1)],
                    ident[0:65, 0:65])
            ptv = pts[hh][:, 0:264].rearrange("p (j e) -> p j e", e=66)
            ptvs.append(ptv)
            nc.vector.reciprocal(rcs[hh][:], ptv[:, :, 64])
        for j in range(4):
            for hh in range(2):
                h = 2 + hh
                osl = outp[g][:, 256 * j + 64 * h:256 * j + 64 * (h + 1)]
                if hh == 1:
                    nc.scalar.activation(
                        osl, ptvs[hh][:, j, 0:64],
                        mybir.ActivationFunctionType.Identity,
                        scale=rcs[hh][:, j:j + 1])
                    nc.gpsimd.tensor_add(osl, osl,
                                         bvb[:, 64 * h:64 * (h + 1)])
                else:
                    nc.vector.scalar_tensor_tensor(
                        osl, ptvs[hh][:, j, 0:64], rcs[hh][:, j:j + 1],
                        bvb[:, 64 * h:64 * (h + 1)], op0=MUL, op1=ADD)
            if j % 2 == 1:
                nc.sync.dma_start(
                    d["out"][512 * g + 128 * (j - 1):
                             512 * g + 128 * (j + 1), :]
                    .rearrange("(j p) f -> p j f", p=128),
                    outp[g][:, 256 * (j - 1):256 * (j + 1)]
                    .rearrange("p (j f) -> p j f", j=2))


_NC_CACHE = {}


def _build(s_kv):
    if s_kv in _NC_CACHE:
        return _NC_CACHE[s_kv]
    nc = bass.Bass(trn_type="TRN2", target_bir_lowering=False, debug=False)
    d = {
        "xqT": nc.dram_tensor("xqT", [H, S], MMDT, kind="ExternalInput").ap(),
        "xkT": nc.dram_tensor("xkT", [H, s_kv], MMDT,
                              kind="ExternalInput").ap(),
        "xvT": nc.dram_tensor("xvT", [H, s_kv], MMDT,
                              kind="ExternalInput").ap(),
        "wqT": nc.dram_tensor("wqT", [H, F], MMDT, kind="ExternalInput").ap(),
        "wkT": nc.dram_tensor("wkT", [H, F], MMDT, kind="ExternalInput").ap(),
        "wvT": nc.dram_tensor("wvT", [H, F], MMDT, kind="ExternalInput").ap(),
        "bqr": nc.dram_tensor("bqr", [128, 2], F32, kind="ExternalInput").ap(),
        "bkr": nc.dram_tensor("bkr", [128, 2], F32, kind="ExternalInput").ap(),
        "bvr": nc.dram_tensor("bvr", [1, F], F32, kind="ExternalInput").ap(),
        "mbias": nc.dram_tensor("mbias", [128, 1], F32,
                                kind="ExternalInput").ap(),
        "out": nc.dram_tensor("out", [S, F], MMDT, kind="ExternalOutput").ap(),
    }
    with tile.TileContext(nc) as tc:
        _emit(nc, tc, d, s_kv)
    _legalize_sync(nc)
    _NC_CACHE[s_kv] = nc
    return nc


def plan_kv(mask):
    """Per-batch compaction plan: indices of valid key positions and the
    padded kv length shared across batches (multiple of 128)."""
    mask = np.asarray(mask)
    idxs = [np.nonzero(mask[b])[0] for b in range(B)]
    nmax = max((len(i) for i in idxs), default=1)
    s_kv = min(S, max(128, -(-nmax // 128) * 128))
    return idxs, s_kv


def make_in_maps(query, key, value, mask, Wq, bq, Wk, bk, Wv, bv,
                 idxs=None, s_kv=None):
    import ml_dtypes
    bf16 = ml_dtypes.bfloat16
    if idxs is None:
        idxs, s_kv = plan_kv(mask)
    query, key, value = (np.asarray(a, np.float32)
                         for a in (query, key, value))
    Wq, Wk, Wv = (np.asarray(a, np.float32) for a in (Wq, Wk, Wv))
    bq, bk, bv = (np.asarray(a, np.float32) for a in (bq, bk, bv))
    in_maps = []
    qc, kc, vc, mbc = {}, {}, {}, {}
    for b in range(B):
        idx = idxs[b]
        qc[b] = np.ascontiguousarray(query[b].T.astype(bf16))
        kcb = np.zeros((H, s_kv), bf16)
        kcb[:, :len(idx)] = key[b][idx].T.astype(bf16)
        vcb = np.zeros((H, s_kv), bf16)
        vcb[:, :len(idx)] = value[b][idx].T.astype(bf16)
        # per-partition bias column for the LAST kv tile only
        mb = np.full(128, NEG, np.float32)
        nlast = len(idx) - (s_kv - 128)
        if nlast > 0:
            mb[:nlast] = 0.0
        kc[b], vc[b] = kcb, vcb
        mbc[b] = np.ascontiguousarray(mb.reshape(128, 1))
    for c in range(N_CORES):
        b = c // (N_CORES // B)
        fs = F * (c % (N_CORES // B))
        in_maps.append({
            "xqT": qc[b],
            "xkT": kc[b],
            "xvT": vc[b],
            "wqT": np.ascontiguousarray(Wq[fs:fs + F].T.astype(bf16)),
            "wkT": np.ascontiguousarray(Wk[fs:fs + F].T.astype(bf16)),
            "wvT": np.ascontiguousarray(Wv[fs:fs + F].T.astype(bf16)),
            "bqr": np.ascontiguousarray(bq[fs:fs + F].reshape(2, 128).T),
            "bkr": np.ascontiguousarray(bk[fs:fs + F].reshape(2, 128).T),
            "bvr": np.ascontiguousarray(bv[fs:fs + F].reshape(1, F)),
            "mbias": mbc[b],
        })
    return in_maps


def assemble(results):
    out = np.empty((B, S, H), np.float32)
    for c in range(N_CORES):
        b = c // (N_CORES // B)
        fs = F * (c % (N_CORES // B))
        out[b, :, fs:fs + F] = np.asarray(results[c]["out"],
                                          dtype=np.float32)
    return out


def kernel(query, key, value, mask, Wq, bq, Wk, bk, Wv, bv, _trace=False):
    idxs, s_kv = plan_kv(mask)
    nc = _build(s_kv)
    in_maps = make_in_maps(query, key, value, mask, Wq, bq, Wk, bk, Wv, bv,
                           idxs, s_kv)
    res = run_bass_kernel_spmd(nc, in_maps, core_ids=list(range(N_CORES)),
                               trace=_trace)
    out = assemble(res.results)
    if _trace:
        return out, res
    return out
